# revision 36
# baseline (speedup 1.0000x reference)
"""Multi-head attention (L=2048, EMB=1024, H=16, D=64) on 8 TRN2 NeuronCores.

Tensor-parallel over heads: core i owns heads {2i, 2i+1} (a 128-row block of
Wq/Wk/Wv and a 128-column block of Wo). Each core computes its two heads'
attention plus its partial output projection; the host sums the 8 partials.

Device-side layout is fully transposed (scores^T = [m, l]) so no on-device
transposes are needed:
  QT[d, l] = (Wq_shard @ q^T)        lhsT = (Wq_shard/8)^T, rhs = q^T
  KT[d, l] = (Wk_shard @ k^T)
  V [m, d] = (v @ Wv_shard^T)        lhsT = v^T tile,       rhs = Wv_shard^T
  sT[m, l] = KT_h^T @ QT_h  -  60*mask   (mask applied ON THE PE: a second
             matmul with stationary diag(-60) fp8 and the fp8 {0,1} mask tile
             as the moving operand accumulates into the same PSUM bank, so
             exp(sT) is already masked and no DVE mask-multiply is needed)
  pT       = exp(sT)                 (no max-subtraction: |s| <~ 9)
  attnT/Z  = [V_h | 1]^T @ pT        (ones column gives softmax denominator)
  outT     = Wo_shard^T-block @ (attnT / Z)   partial (bf16), summed on host

All matmuls run in bf16/fp8 (fp32 PSUM accumulation).

Pipeline structure (tuned against neuron-profile NTFF traces):
- 12 warmup matmuls on zeroed SBUF ramp the PE HAM p-state to full clock
  while the first input DMAs land.
- the scalar HWDGE ring carries ONLY wq,q0,wk,k0 so the first projection
  starts ~10us in; mask chunks ride the gpsimd SWDGE ring; everything else
  is on sync; output stores alternate sync/vector.
- l-tile-major attention; each l-tile's epilogue (softmax-denominator
  reciprocal + broadcast + normalize + output projection + store) is
  deferred and drip-fed as filler into the NEXT l-tile's quad stream.
- the softmax denominator never leaves SBUF: DVE reciprocal on the [2, 512]
  Z rows, then one SBUF->SBUF broadcast DMA fans 1/Z out to 128 partitions.
- exp() is batched over 3 key-tiles per ACTIVATE.
- One-stage software pipeline on the PE queue: quad q's attn matmuls are
  emitted after quad q+1's scores, decoupling PE from the exp chain.
- K/V projection rounds are interleaved into the first head's attention
  stream; Q chunk projections fill the second head's stream.
"""

import sys

for _p in ("/opt/trn_rl_repo",):
    if _p not in sys.path:
        sys.path.insert(0, _p)

from contextlib import ExitStack

import ml_dtypes
import numpy as np

import concourse.bass as bass
import concourse.tile as tile
from concourse import bacc, library_config, mybir
from concourse._compat import with_exitstack
from concourse.bass_utils import run_bass_kernel_spmd

BF16 = mybir.dt.bfloat16
FP8 = mybir.dt.float8e4
F32 = mybir.dt.float32
NPBF16 = ml_dtypes.bfloat16
NPFP8 = ml_dtypes.float8_e4m3

L = 2048
EMB = 1024
NHEAD = 16
HEAD_DIM = 64
NCORES = 8
HPC = NHEAD // NCORES  # heads per core = 2
ROWS = HPC * HEAD_DIM  # weight rows per core = 128
SCALE = HEAD_DIM ** -0.5
MASKNEG = -60.0

LT = 512               # l-tile (matmul free dim / PSUM bank)
NLT = L // LT          # 4
MT = 128               # m-tile (key-block on partitions)
NMT = L // MT          # 16
ET = 128               # contraction tile over EMB
NET = EMB // ET        # 8
JT = 128               # output-row tile
NJT = EMB // JT        # 8

QUADS = (3, 3, 3, 3, 2, 2)   # m-tiles per exp instruction
QB = 3                        # psc tile m-capacity (PSUM banks per slot)
PSC_BUFS = 2
WARMUP = 28                   # p-state ramp matmuls before real work
MASK8 = True                  # mask as fp8 (else bf16)
MDT = FP8 if MASK8 else BF16
NPMDT = NPFP8 if MASK8 else NPBF16


@with_exitstack
def _mha_kernel(ctx, tc, outT, qT, kT, vT, wqT, wkT, wvT, woT, maskT, diagT):
    nc = tc.nc

    const = ctx.enter_context(tc.tile_pool(name="const", bufs=1))
    ppool = ctx.enter_context(tc.tile_pool(name="ptiles", bufs=5))
    maskp = ctx.enter_context(tc.tile_pool(name="maskp", bufs=3))
    stage = ctx.enter_context(tc.tile_pool(name="stage", bufs=4))
    zpool = ctx.enter_context(tc.tile_pool(name="zpool", bufs=2))
    psc = ctx.enter_context(tc.tile_pool(name="psc", bufs=PSC_BUFS, space="PSUM"))
    psa = ctx.enter_context(tc.tile_pool(name="psa", bufs=1, space="PSUM"))
    outp = ctx.enter_context(tc.tile_pool(name="outp", bufs=1, space="PSUM"))

    # ---- resident input tiles; DMAs emitted in consumption order ----
    qTs = const.tile([128, NET, L], BF16, tag="qTs")
    kTs = const.tile([128, NET, L], BF16, tag="kTs")
    vTs = const.tile([128, NET, L], BF16, tag="vTs")
    wqs = const.tile([128, NET, ROWS], BF16, tag="wqs")
    wks = const.tile([128, NET, ROWS], BF16, tag="wks")
    wvs = const.tile([128, NET, ROWS], BF16, tag="wvs")
    wos = const.tile([128, EMB], BF16, tag="wos")  # [hd, j]
    diag = const.tile([128, 128], MDT, tag="diag")
    q3 = qT.rearrange("(o p) l -> p o l", p=128)
    k3 = kT.rearrange("(o p) l -> p o l", p=128)
    v3 = vT.rearrange("(o p) l -> p o l", p=128)

    def chunk_dma_eng(eng, dst, src3, lc):
        eng.dma_start(dst[:, :, bass.ts(lc, LT)], src3[:, :, bass.ts(lc, LT)])

    state = {}
    mask3 = maskT.rearrange("h (mo p) l -> h p mo l", p=128)

    def mask_fetch(lt, h, eng=None):
        mc = maskp.tile([128, NMT, LT], MDT, tag="maskc", name=f"maskc_{lt}_{h}")
        (eng or nc.sync).dma_start(mc[:], mask3[h, :, :, bass.ts(lt, LT)])
        state[lt, h, "maskc"] = mc

    # critical first chunks alone and FIRST on the sync HWDGE ring; the
    # HBM drains roughly in global issue order, so nothing else may be
    # in flight before these.
    nc.sync.dma_start(wqs[:], wqT[:])
    chunk_dma_eng(nc.sync, qTs, q3, 0)
    nc.sync.dma_start(wks[:], wkT[:])
    chunk_dma_eng(nc.sync, kTs, k3, 0)
    nc.sync.dma_start(wvs[:], wvT[:])
    chunk_dma_eng(nc.sync, vTs, v3, 0)
    # Everything else follows on the same ring in consumption order: the
    # ring admits ~4 DMAs in flight, so issue self-paces to completions
    # and the critical chunks above are never crowded out of HBM.
    nc.sync.dma_start(diag[:], diagT[:])
    mask_fetch(0, 0, eng=nc.sync)
    chunk_dma_eng(nc.sync, kTs, k3, 1)
    chunk_dma_eng(nc.sync, vTs, v3, 1)
    mask_fetch(0, 1)
    chunk_dma_eng(nc.sync, kTs, k3, 2)
    chunk_dma_eng(nc.sync, vTs, v3, 2)
    chunk_dma_eng(nc.sync, kTs, k3, 3)
    chunk_dma_eng(nc.sync, vTs, v3, 3)
    chunk_dma_eng(nc.sync, qTs, q3, 1)
    chunk_dma_eng(nc.sync, qTs, q3, 2)
    chunk_dma_eng(nc.sync, qTs, q3, 3)
    nc.sync.dma_start(wos[:], woT[:])

    QTb = const.tile([128, L], BF16, tag="QTb")
    KTb = const.tile([128, L], BF16, tag="KTb")
    VROW = 66
    vaug = const.tile([128, HPC, NMT, VROW], BF16, tag="vaug")
    nc.vector.memset(vaug[:, :, :, HEAD_DIM : HEAD_DIM + 1], 1.0)
    nc.vector.memset(vaug[:, :, :, HEAD_DIM + 1 : VROW], 0.0)

    # ---- PE p-state warmup: matmuls on zeroed SBUF while DMAs land ----
    if WARMUP:
        wstat = const.tile([128, 128], BF16, tag="wstat")
        wmov = const.tile([128, LT], BF16, tag="wmov")
        nc.vector.memset(wstat[:], 0.0)
        nc.vector.memset(wmov[:], 0.0)
        pw = psc.tile([128, QB, LT], F32, tag="psc", name="ps_warm")[:, 0, :]
        for i in range(WARMUP):
            nc.tensor.matmul(pw[:], lhsT=wstat[:], rhs=wmov[:],
                             start=(i == 0), stop=(i == WARMUP - 1))

    def qk_proj(dst, w, x, lt):
        ps = psc.tile([128, QB, LT], F32, tag="psc", name="ps_proj")[:, 0, :]
        for et in range(NET):
            nc.tensor.matmul(
                ps[:],
                lhsT=w[:, et, :],
                rhs=x[:, et, bass.ts(lt, LT)],
                start=(et == 0),
                stop=(et == NET - 1),
            )
        nc.vector.tensor_copy(out=dst[:, bass.ts(lt, LT)], in_=ps[:])

    def v_proj(mt):
        ps = psc.tile([128, QB, LT], F32, tag="psc", name="ps_v")[:, 0, :ROWS]
        for et in range(NET):
            nc.tensor.matmul(
                ps[:],
                lhsT=vTs[:, et, bass.ts(mt, MT)],
                rhs=wvs[:, et, :],
                start=(et == 0),
                stop=(et == NET - 1),
            )
        for h in range(HPC):
            nc.vector.tensor_copy(
                out=vaug[:, h, mt, 0:HEAD_DIM],
                in_=ps[:, bass.ts(h, HEAD_DIM)],
            )

    # ---- attention + per-l-tile epilogue ----
    attnTb = const.tile([128, L], BF16, tag="attnTb")

    # Epilogue work for l-tile X is deferred and drip-fed as PE/DVE filler
    # into l-tile X+1's quad stream.
    pending = []
    final_drain = [False]

    zdramb = nc.dram_tensor("zdramb", [NLT, HPC, LT], F32).ap()

    def piece_recip(lt, h):
        def go():
            zrow = state[lt, h, "zrow"]
            zinv = zpool.tile([1, LT], F32, tag="zinv", name=f"zinv_{lt}_{h}")
            nc.vector.reciprocal_approx_fast(zinv[:], zrow[:])
            nc.sync.dma_start(zdramb[lt, h][None, :], zinv[:])
        return go

    def piece_bcast(lt, h):
        def go():
            if h == 0:
                zinvb = zpool.tile([128, LT], F32, tag="zinvb",
                                   name=f"zinvb_{lt}")
                state[lt, "zinvb"] = zinvb
            zinvb = state[lt, "zinvb"]
            nc.sync.dma_start(
                zinvb[bass.ts(h, HEAD_DIM), :],
                zdramb[lt, h][None, :].to_broadcast((HEAD_DIM, LT)),
            )
        return go

    def piece_norm(lt):
        def go():
            ls = bass.ts(lt, LT)
            nc.vector.tensor_mul(
                out=attnTb[:, ls], in0=attnTb[:, ls], in1=state[lt, "zinvb"][:]
            )
        return go

    def piece_outproj(lt, jt):
        def go():
            ls = bass.ts(lt, LT)
            if final_drain[0]:
                ps = psc.tile([128, QB, LT], F32, tag="psc", name="ps_out")[:, 0, :]
            else:
                ps = outp.tile([128, LT], F32, tag="outp", name="ps_out")
            nc.tensor.matmul(
                ps[:],
                lhsT=wos[:, bass.ts(jt, JT)],
                rhs=attnTb[:, ls],
                start=True,
                stop=True,
            )
            st = stage.tile([128, LT], BF16, tag="st", name="st")
            if final_drain[0] and jt % 2:
                nc.scalar.copy(out=st[:], in_=ps[:])
            else:
                nc.vector.tensor_copy(out=st[:], in_=ps[:])
            eng = nc.gpsimd if jt % 2 else nc.sync
            eng.dma_start(outT[bass.ts(jt, JT), ls], st[:])
        return go

    qk_proj(QTb, wqs, qTs, 0)

    for lt in range(NLT):
        ls = bass.ts(lt, LT)
        for h in range(HPC):
            hd = bass.ts(h, HEAD_DIM)
            for nxt in (lt * HPC + h + 1, lt * HPC + h + 2):
                if nxt < NLT * HPC:
                    nl, nh = nxt // HPC, nxt % HPC
                    if (nl, nh, "maskc") not in state:
                        mask_fetch(nl, nh)
            maskc = state[lt, h, "maskc"]
            pa = psa.tile([128, LT], F32, tag="psa", name=f"psa_{lt}_{h}")
            mt0 = 0
            attn_queue = []
            for qi, qn in enumerate(QUADS):
                if lt == 0 and h == 0:
                    # interleave K/V projection rounds into the first
                    # attention stream so the PE queue never drains
                    for mt in range(mt0, mt0 + qn):
                        if mt % (LT // MT) == 0:
                            qk_proj(KTb, wks, kTs, mt // (LT // MT))
                        v_proj(mt)
                if lt == 0 and h == 1 and 1 <= qi <= 3:
                    qk_proj(QTb, wqs, qTs, qi)  # PE filler + needed later
                budget = 2 if len(pending) > 6 else 1
                while pending and budget:
                    pending[0][0] -= 1
                    if pending[0][0] < 0:
                        pending.pop(0)[1]()
                        budget -= 1
                    else:
                        break
                ss = psc.tile([128, QB, LT], F32, tag="psc", name="ss")
                for i in range(qn):
                    mt = mt0 + i
                    nc.tensor.matmul(
                        ss[:, i, :],
                        lhsT=KTb[hd, bass.ts(mt, MT)],
                        rhs=QTb[hd, ls],
                        start=True,
                        stop=False,
                        skip_group_check=True,
                    )
                # additive mask: ss += diag(-60).T @ mask; grouped after
                # the scores so the diag stationary stays resident and the
                # operand dtype doesn't flip per matmul
                for i in range(qn):
                    nc.tensor.matmul(
                        ss[:, i, :],
                        lhsT=diag[:],
                        rhs=maskc[:, mt0 + i, :],
                        start=False,
                        stop=True,
                        skip_group_check=True,
                    )
                # two-stage software pipeline on PE: quad q-2's attn
                # matmuls are emitted after quad q's scores, so exp(q-2)
                # has two quads of PE time to land and the PE queue never
                # waits on the scalar engine mid-stream
                if len(attn_queue) >= 2:
                    attn_queue.pop(0)()
                pT = ppool.tile([128, QB, LT], BF16, tag="pT", name="pT")
                nc.scalar.activation(
                    pT[:, :qn, :], ss[:, :qn, :], mybir.ActivationFunctionType.Exp
                )

                def make_attn(mt0=mt0, qn=qn, pT=pT):
                    def go():
                        for i in range(qn):
                            mt = mt0 + i
                            nc.tensor.matmul(
                                pa[:VROW, :],
                                lhsT=vaug[:, h, mt, :],
                                rhs=pT[:, i, :],
                                start=(mt == 0),
                                stop=(mt == NMT - 1),
                            )
                    return go

                attn_queue.append(make_attn())
                mt0 += qn
            while attn_queue:
                attn_queue.pop(0)()
            nc.vector.tensor_copy(out=attnTb[hd, ls], in_=pa[0:HEAD_DIM, :])
            zrow = zpool.tile([1, LT], F32, tag="zrow", name=f"zrow_{lt}_{h}")
            state[lt, h, "zrow"] = zrow
            nc.vector.tensor_copy(
                out=zrow[0:1, :],
                in_=pa[HEAD_DIM : HEAD_DIM + 1, :],
            )
            pending.append([0, piece_recip(lt, h)])
            pending.append([1, piece_bcast(lt, h)])
        pending.append([2, piece_norm(lt)])
        for jt in range(NJT):
            pending.append([1 if jt == 0 else 0, piece_outproj(lt, jt)])

    final_drain[0] = True
    while pending:
        pending.pop(0)[1]()


_CACHE = {}


def _build():
    key = "nc"
    if key in _CACHE:
        return _CACHE[key]
    nc = bacc.Bacc("TRN2", target_bir_lowering=False, debug=False,
                   num_devices=NCORES)
    qT = nc.dram_tensor("qT", [EMB, L], BF16, kind="ExternalInput").ap()
    kT = nc.dram_tensor("kT", [EMB, L], BF16, kind="ExternalInput").ap()
    vT = nc.dram_tensor("vT", [EMB, L], BF16, kind="ExternalInput").ap()
    wqT = nc.dram_tensor("wqT", [128, NET, ROWS], BF16, kind="ExternalInput").ap()
    wkT = nc.dram_tensor("wkT", [128, NET, ROWS], BF16, kind="ExternalInput").ap()
    wvT = nc.dram_tensor("wvT", [128, NET, ROWS], BF16, kind="ExternalInput").ap()
    woT = nc.dram_tensor("woT", [ROWS, EMB], BF16, kind="ExternalInput").ap()
    maskT = nc.dram_tensor("maskT", [HPC, L, L], MDT, kind="ExternalInput").ap()
    diagT = nc.dram_tensor("diagT", [128, 128], MDT, kind="ExternalInput").ap()
    outT = nc.dram_tensor("outT", [EMB, L], BF16, kind="ExternalOutput").ap()

    with tile.TileContext(nc) as tc:
        _mha_kernel(tc, outT, qT, kT, vT, wqT, wkT, wvT, woT, maskT, diagT)
    nc.compile()
    _CACHE[key] = nc
    return nc


def _pack_w(w, npdt):
    # [ROWS, EMB] -> w.T [EMB, ROWS] -> [128, NET, ROWS] with e = o*128+p
    return np.ascontiguousarray(
        w.T.reshape(NET, 128, ROWS).transpose(1, 0, 2)
    ).astype(npdt)


def _prep_in_maps(q, k, v, mask, Wq, Wk, Wv, Wo):
    qT = np.ascontiguousarray(q.T).astype(NPBF16)
    kT = np.ascontiguousarray(k.T).astype(NPBF16)
    vT = np.ascontiguousarray(v.T).astype(NPBF16)
    diagT = (np.eye(128, dtype=np.float32) * MASKNEG).astype(NPMDT)
    in_maps = []
    for c in range(NCORES):
        rows = slice(c * ROWS, (c + 1) * ROWS)
        in_maps.append({
            "qT": qT,
            "kT": kT,
            "vT": vT,
            "wqT": _pack_w(Wq[rows] * SCALE, NPBF16),
            "wkT": _pack_w(Wk[rows], NPBF16),
            "wvT": _pack_w(Wv[rows], NPBF16),
            "woT": np.ascontiguousarray(Wo[:, rows].T).astype(NPBF16),
            "maskT": np.ascontiguousarray(
                mask[c * HPC : (c + 1) * HPC].swapaxes(1, 2)
            ).astype(NPMDT),
            "diagT": diagT,
        })
    return in_maps


def run(q, k, v, mask, Wq, Wk, Wv, Wo, **spmd_kwargs):
    nc = _build()
    in_maps = _prep_in_maps(q, k, v, mask, Wq, Wk, Wv, Wo)
    res = run_bass_kernel_spmd(nc, in_maps, list(range(NCORES)), **spmd_kwargs)
    outT = np.zeros((EMB, L), np.float64)
    for r in res.results:
        outT += r["outT"].astype(np.float64)
    out = np.ascontiguousarray(outT.T).astype(np.float32)
    return out, res


def kernel(q, k, v, mask, Wq, Wk, Wv, Wo):
    q, k, v = (np.asarray(x, np.float32) for x in (q, k, v))
    Wq, Wk, Wv, Wo = (np.asarray(x, np.float32) for x in (Wq, Wk, Wv, Wo))
    mask = np.asarray(mask, bool)
    out, _ = run(q, k, v, mask, Wq, Wk, Wv, Wo)
    return out


# revision 37
# speedup vs baseline: 1.1466x; 1.1466x over previous
"""Multi-head attention (L=2048, EMB=1024, H=16, D=64) on 8 TRN2 NeuronCores.

Tensor-parallel over heads: core i owns heads {2i, 2i+1} (a 128-row block of
Wq/Wk/Wv and a 128-column block of Wo). Each core computes its two heads'
attention plus its partial output projection; the host sums the 8 partials.

Device-side layout is fully transposed (scores^T = [m, l]) so no on-device
transposes are needed:
  QT[d, l] = (Wq_shard @ q^T)        lhsT = (Wq_shard/8)^T, rhs = q^T
  KT[d, l] = (Wk_shard @ k^T)
  V [m, d] = (v @ Wv_shard^T)        lhsT = v^T tile,       rhs = Wv_shard^T
  sT[m, l] = KT_h^T @ QT_h  -  60*mask   (mask applied ON THE PE: a second
             matmul with stationary diag(-60) fp8 and the fp8 {0,1} mask tile
             as the moving operand accumulates into the same PSUM bank, so
             exp(sT) is already masked and no DVE mask-multiply is needed)
  pT       = exp(sT)                 (no max-subtraction: |s| <~ 9)
  attnT/Z  = [V_h | 1]^T @ pT        (ones column gives softmax denominator)
  outT     = Wo_shard^T-block @ (attnT / Z)   partial (bf16), summed on host

All matmuls run in bf16/fp8 (fp32 PSUM accumulation).

Pipeline structure (tuned against neuron-profile NTFF traces):
- 12 warmup matmuls on zeroed SBUF ramp the PE HAM p-state to full clock
  while the first input DMAs land.
- the scalar HWDGE ring carries ONLY wq,q0,wk,k0 so the first projection
  starts ~10us in; mask chunks ride the gpsimd SWDGE ring; everything else
  is on sync; output stores alternate sync/vector.
- l-tile-major attention; each l-tile's epilogue (softmax-denominator
  reciprocal + broadcast + normalize + output projection + store) is
  deferred and drip-fed as filler into the NEXT l-tile's quad stream.
- the softmax denominator never leaves SBUF: DVE reciprocal on the [2, 512]
  Z rows, then one SBUF->SBUF broadcast DMA fans 1/Z out to 128 partitions.
- exp() is batched over 3 key-tiles per ACTIVATE.
- One-stage software pipeline on the PE queue: quad q's attn matmuls are
  emitted after quad q+1's scores, decoupling PE from the exp chain.
- K/V projection rounds are interleaved into the first head's attention
  stream; Q chunk projections fill the second head's stream.
"""

import sys

for _p in ("/opt/trn_rl_repo",):
    if _p not in sys.path:
        sys.path.insert(0, _p)

from contextlib import ExitStack

import ml_dtypes
import numpy as np

import concourse.bass as bass
import concourse.tile as tile
from concourse import bacc, library_config, mybir
from concourse._compat import with_exitstack
from concourse.bass_utils import run_bass_kernel_spmd

BF16 = mybir.dt.bfloat16
FP8 = mybir.dt.float8e4
F32 = mybir.dt.float32
NPBF16 = ml_dtypes.bfloat16
NPFP8 = ml_dtypes.float8_e4m3

L = 2048
EMB = 1024
NHEAD = 16
HEAD_DIM = 64
NCORES = 8
HPC = NHEAD // NCORES  # heads per core = 2
ROWS = HPC * HEAD_DIM  # weight rows per core = 128
SCALE = HEAD_DIM ** -0.5
MASKNEG = -60.0

LT = 512               # l-tile (matmul free dim / PSUM bank)
NLT = L // LT          # 4
MT = 128               # m-tile (key-block on partitions)
NMT = L // MT          # 16
ET = 128               # contraction tile over EMB
NET = EMB // ET        # 8
JT = 128               # output-row tile
NJT = EMB // JT        # 8

QUADS = (3, 3, 3, 3, 2, 2)   # m-tiles per exp instruction
QB = 3                        # psc tile m-capacity (PSUM banks per slot)
PSC_BUFS = 2
WARMUP = 28                   # p-state ramp matmuls before real work
MASK8 = True                  # mask as fp8 (else bf16)
MDT = FP8 if MASK8 else BF16
NPMDT = NPFP8 if MASK8 else NPBF16


@with_exitstack
def _mha_kernel(ctx, tc, outT, qT, kT, vT, wqT, wkT, wvT, woT, maskT, diagT):
    nc = tc.nc

    const = ctx.enter_context(tc.tile_pool(name="const", bufs=1))
    ppool = ctx.enter_context(tc.tile_pool(name="ptiles", bufs=5))
    maskp = ctx.enter_context(tc.tile_pool(name="maskp", bufs=3))
    stage = ctx.enter_context(tc.tile_pool(name="stage", bufs=4))
    zpool = ctx.enter_context(tc.tile_pool(name="zpool", bufs=2))
    psc = ctx.enter_context(tc.tile_pool(name="psc", bufs=PSC_BUFS, space="PSUM"))
    psa = ctx.enter_context(tc.tile_pool(name="psa", bufs=1, space="PSUM"))
    outp = ctx.enter_context(tc.tile_pool(name="outp", bufs=1, space="PSUM"))

    # ---- resident input tiles; DMAs emitted in consumption order ----
    qTs = const.tile([128, NET, L], BF16, tag="qTs")
    kTs = const.tile([128, NET, L], BF16, tag="kTs")
    vTs = const.tile([128, NET, L], BF16, tag="vTs")
    wqs = const.tile([128, NET, ROWS], BF16, tag="wqs")
    wks = const.tile([128, NET, ROWS], BF16, tag="wks")
    wvs = const.tile([128, NET, ROWS], BF16, tag="wvs")
    wos = const.tile([128, EMB], BF16, tag="wos")  # [hd, j]
    diag = const.tile([128, 128], MDT, tag="diag")
    q3 = qT.rearrange("(o p) l -> p o l", p=128)
    k3 = kT.rearrange("(o p) l -> p o l", p=128)
    v3 = vT.rearrange("(o p) l -> p o l", p=128)

    def chunk_dma_eng(eng, dst, src3, lc):
        eng.dma_start(dst[:, :, bass.ts(lc, LT)], src3[:, :, bass.ts(lc, LT)])

    state = {}
    mask3 = maskT.rearrange("h (mo p) l -> h p mo l", p=128)

    def mask_fetch(lt, h, eng=None):
        mc = maskp.tile([128, NMT, LT], MDT, tag="maskc", name=f"maskc_{lt}_{h}")
        (eng or nc.sync).dma_start(mc[:], mask3[h, :, :, bass.ts(lt, LT)])
        state[lt, h, "maskc"] = mc

    # critical first chunks alone and FIRST on the sync HWDGE ring; the
    # HBM drains roughly in global issue order, so nothing else may be
    # in flight before these.
    nc.sync.dma_start(wqs[:], wqT[:])
    chunk_dma_eng(nc.sync, qTs, q3, 0)
    nc.sync.dma_start(wks[:], wkT[:])
    chunk_dma_eng(nc.sync, kTs, k3, 0)
    nc.sync.dma_start(wvs[:], wvT[:])
    chunk_dma_eng(nc.sync, vTs, v3, 0)
    # Everything else follows on the same ring in consumption order: the
    # ring admits ~4 DMAs in flight, so issue self-paces to completions
    # and the critical chunks above are never crowded out of HBM.
    nc.sync.dma_start(diag[:], diagT[:])
    mask_fetch(0, 0, eng=nc.sync)
    chunk_dma_eng(nc.sync, kTs, k3, 1)
    chunk_dma_eng(nc.sync, vTs, v3, 1)
    mask_fetch(0, 1)
    chunk_dma_eng(nc.sync, kTs, k3, 2)
    chunk_dma_eng(nc.sync, vTs, v3, 2)
    chunk_dma_eng(nc.sync, kTs, k3, 3)
    chunk_dma_eng(nc.sync, vTs, v3, 3)
    chunk_dma_eng(nc.sync, qTs, q3, 1)
    chunk_dma_eng(nc.sync, qTs, q3, 2)
    chunk_dma_eng(nc.sync, qTs, q3, 3)
    nc.sync.dma_start(wos[:], woT[:])

    QTb = const.tile([128, L], BF16, tag="QTb")
    KTb = const.tile([128, L], BF16, tag="KTb")
    VROW = 66
    vaug = const.tile([128, HPC, NMT, VROW], BF16, tag="vaug")
    nc.vector.memset(vaug[:, :, :, HEAD_DIM : HEAD_DIM + 1], 1.0)
    nc.vector.memset(vaug[:, :, :, HEAD_DIM + 1 : VROW], 0.0)

    # ---- PE p-state warmup: matmuls on zeroed SBUF while DMAs land ----
    if WARMUP:
        wstat = const.tile([128, 128], BF16, tag="wstat")
        wmov = const.tile([128, LT], BF16, tag="wmov")
        nc.vector.memset(wstat[:], 0.0)
        nc.vector.memset(wmov[:], 0.0)
        pw = psc.tile([128, QB, LT], F32, tag="psc", name="ps_warm")[:, 0, :]
        for i in range(WARMUP):
            nc.tensor.matmul(pw[:], lhsT=wstat[:], rhs=wmov[:],
                             start=(i == 0), stop=(i == WARMUP - 1))

    def qk_proj(dst, w, x, lt):
        ps = psc.tile([128, QB, LT], F32, tag="psc", name="ps_proj")[:, 0, :]
        for et in range(NET):
            nc.tensor.matmul(
                ps[:],
                lhsT=w[:, et, :],
                rhs=x[:, et, bass.ts(lt, LT)],
                start=(et == 0),
                stop=(et == NET - 1),
            )
        nc.vector.tensor_copy(out=dst[:, bass.ts(lt, LT)], in_=ps[:])

    def v_proj(mt):
        ps = psc.tile([128, QB, LT], F32, tag="psc", name="ps_v")[:, 0, :ROWS]
        for et in range(NET):
            nc.tensor.matmul(
                ps[:],
                lhsT=vTs[:, et, bass.ts(mt, MT)],
                rhs=wvs[:, et, :],
                start=(et == 0),
                stop=(et == NET - 1),
            )
        for h in range(HPC):
            nc.vector.tensor_copy(
                out=vaug[:, h, mt, 0:HEAD_DIM],
                in_=ps[:, bass.ts(h, HEAD_DIM)],
            )

    # ---- attention + per-l-tile epilogue ----
    attnTb = const.tile([128, L], BF16, tag="attnTb")

    # Epilogue work for l-tile X is deferred and drip-fed as PE/DVE filler
    # into l-tile X+1's quad stream.
    pending = []
    final_drain = [False]

    zdramb = nc.dram_tensor("zdramb", [NLT, HPC, LT], F32).ap()

    def piece_recip(lt, h):
        def go():
            zrow = state[lt, h, "zrow"]
            zinv = zpool.tile([1, LT], F32, tag="zinv", name=f"zinv_{lt}_{h}")
            nc.vector.reciprocal_approx_fast(zinv[:], zrow[:])
            nc.scalar.dma_start(zdramb[lt, h][None, :], zinv[:])
        return go

    def piece_bcast(lt, h):
        def go():
            if h == 0:
                zinvb = zpool.tile([128, LT], F32, tag="zinvb",
                                   name=f"zinvb_{lt}")
                state[lt, "zinvb"] = zinvb
            zinvb = state[lt, "zinvb"]
            nc.scalar.dma_start(
                zinvb[bass.ts(h, HEAD_DIM), :],
                zdramb[lt, h][None, :].to_broadcast((HEAD_DIM, LT)),
            )
        return go

    def piece_norm(lt):
        def go():
            ls = bass.ts(lt, LT)
            nc.vector.tensor_mul(
                out=attnTb[:, ls], in0=attnTb[:, ls], in1=state[lt, "zinvb"][:]
            )
        return go

    def piece_outproj(lt, jt):
        def go():
            ls = bass.ts(lt, LT)
            if final_drain[0]:
                ps = psc.tile([128, QB, LT], F32, tag="psc", name="ps_out")[:, 0, :]
            else:
                ps = outp.tile([128, LT], F32, tag="outp", name="ps_out")
            nc.tensor.matmul(
                ps[:],
                lhsT=wos[:, bass.ts(jt, JT)],
                rhs=attnTb[:, ls],
                start=True,
                stop=True,
            )
            st = stage.tile([128, LT], BF16, tag="st", name="st")
            if jt % 2:
                nc.scalar.copy(out=st[:], in_=ps[:])
            else:
                nc.vector.tensor_copy(out=st[:], in_=ps[:])
            eng = nc.scalar if jt % 2 else nc.sync
            eng.dma_start(outT[bass.ts(jt, JT), ls], st[:])
        return go

    qk_proj(QTb, wqs, qTs, 0)

    for lt in range(NLT):
        ls = bass.ts(lt, LT)
        for h in range(HPC):
            hd = bass.ts(h, HEAD_DIM)
            for nxt in (lt * HPC + h + 1, lt * HPC + h + 2):
                if nxt < NLT * HPC:
                    nl, nh = nxt // HPC, nxt % HPC
                    if (nl, nh, "maskc") not in state:
                        mask_fetch(nl, nh)
            maskc = state[lt, h, "maskc"]
            pa = psa.tile([128, LT], F32, tag="psa", name=f"psa_{lt}_{h}")
            mt0 = 0
            attn_queue = []
            for qi, qn in enumerate(QUADS):
                if lt == 0 and h == 0:
                    # interleave K/V projection rounds into the first
                    # attention stream so the PE queue never drains
                    for mt in range(mt0, mt0 + qn):
                        if mt % (LT // MT) == 0:
                            qk_proj(KTb, wks, kTs, mt // (LT // MT))
                        v_proj(mt)
                if lt == 0 and h == 1 and 1 <= qi <= 3:
                    qk_proj(QTb, wqs, qTs, qi)  # PE filler + needed later
                budget = 2 if len(pending) > 6 else 1
                while pending and budget:
                    pending[0][0] -= 1
                    if pending[0][0] < 0:
                        pending.pop(0)[1]()
                        budget -= 1
                    else:
                        break
                ss = psc.tile([128, QB, LT], F32, tag="psc", name="ss")
                for i in range(qn):
                    mt = mt0 + i
                    nc.tensor.matmul(
                        ss[:, i, :],
                        lhsT=KTb[hd, bass.ts(mt, MT)],
                        rhs=QTb[hd, ls],
                        start=True,
                        stop=False,
                        skip_group_check=True,
                    )
                # additive mask: ss += diag(-60).T @ mask; grouped after
                # the scores so the diag stationary stays resident and the
                # operand dtype doesn't flip per matmul
                for i in range(qn):
                    nc.tensor.matmul(
                        ss[:, i, :],
                        lhsT=diag[:],
                        rhs=maskc[:, mt0 + i, :],
                        start=False,
                        stop=True,
                        skip_group_check=True,
                    )
                # two-stage software pipeline on PE: quad q-2's attn
                # matmuls are emitted after quad q's scores, so exp(q-2)
                # has two quads of PE time to land and the PE queue never
                # waits on the scalar engine mid-stream
                if len(attn_queue) >= 2:
                    attn_queue.pop(0)()
                pT = ppool.tile([128, QB, LT], BF16, tag="pT", name="pT")
                nc.scalar.activation(
                    pT[:, :qn, :], ss[:, :qn, :], mybir.ActivationFunctionType.Exp
                )

                def make_attn(mt0=mt0, qn=qn, pT=pT):
                    def go():
                        for i in range(qn):
                            mt = mt0 + i
                            nc.tensor.matmul(
                                pa[:VROW, :],
                                lhsT=vaug[:, h, mt, :],
                                rhs=pT[:, i, :],
                                start=(mt == 0),
                                stop=(mt == NMT - 1),
                            )
                    return go

                attn_queue.append(make_attn())
                mt0 += qn
            while attn_queue:
                attn_queue.pop(0)()
            nc.vector.tensor_copy(out=attnTb[hd, ls], in_=pa[0:HEAD_DIM, :])
            zrow = zpool.tile([1, LT], F32, tag="zrow", name=f"zrow_{lt}_{h}")
            state[lt, h, "zrow"] = zrow
            nc.vector.tensor_copy(
                out=zrow[0:1, :],
                in_=pa[HEAD_DIM : HEAD_DIM + 1, :],
            )
            pending.append([0, piece_recip(lt, h)])
            pending.append([1, piece_bcast(lt, h)])
        pending.append([2, piece_norm(lt)])
        for jt in range(NJT):
            pending.append([1 if jt == 0 else 0, piece_outproj(lt, jt)])

    final_drain[0] = True
    while pending:
        pending.pop(0)[1]()


_CACHE = {}


def _build():
    key = "nc"
    if key in _CACHE:
        return _CACHE[key]
    nc = bacc.Bacc("TRN2", target_bir_lowering=False, debug=False,
                   num_devices=NCORES)
    qT = nc.dram_tensor("qT", [EMB, L], BF16, kind="ExternalInput").ap()
    kT = nc.dram_tensor("kT", [EMB, L], BF16, kind="ExternalInput").ap()
    vT = nc.dram_tensor("vT", [EMB, L], BF16, kind="ExternalInput").ap()
    wqT = nc.dram_tensor("wqT", [128, NET, ROWS], BF16, kind="ExternalInput").ap()
    wkT = nc.dram_tensor("wkT", [128, NET, ROWS], BF16, kind="ExternalInput").ap()
    wvT = nc.dram_tensor("wvT", [128, NET, ROWS], BF16, kind="ExternalInput").ap()
    woT = nc.dram_tensor("woT", [ROWS, EMB], BF16, kind="ExternalInput").ap()
    maskT = nc.dram_tensor("maskT", [HPC, L, L], MDT, kind="ExternalInput").ap()
    diagT = nc.dram_tensor("diagT", [128, 128], MDT, kind="ExternalInput").ap()
    outT = nc.dram_tensor("outT", [EMB, L], BF16, kind="ExternalOutput").ap()

    with tile.TileContext(nc) as tc:
        _mha_kernel(tc, outT, qT, kT, vT, wqT, wkT, wvT, woT, maskT, diagT)
    nc.compile()
    _CACHE[key] = nc
    return nc


def _pack_w(w, npdt):
    # [ROWS, EMB] -> w.T [EMB, ROWS] -> [128, NET, ROWS] with e = o*128+p
    return np.ascontiguousarray(
        w.T.reshape(NET, 128, ROWS).transpose(1, 0, 2)
    ).astype(npdt)


def _prep_in_maps(q, k, v, mask, Wq, Wk, Wv, Wo):
    qT = np.ascontiguousarray(q.T).astype(NPBF16)
    kT = np.ascontiguousarray(k.T).astype(NPBF16)
    vT = np.ascontiguousarray(v.T).astype(NPBF16)
    diagT = (np.eye(128, dtype=np.float32) * MASKNEG).astype(NPMDT)
    in_maps = []
    for c in range(NCORES):
        rows = slice(c * ROWS, (c + 1) * ROWS)
        in_maps.append({
            "qT": qT,
            "kT": kT,
            "vT": vT,
            "wqT": _pack_w(Wq[rows] * SCALE, NPBF16),
            "wkT": _pack_w(Wk[rows], NPBF16),
            "wvT": _pack_w(Wv[rows], NPBF16),
            "woT": np.ascontiguousarray(Wo[:, rows].T).astype(NPBF16),
            "maskT": np.ascontiguousarray(
                mask[c * HPC : (c + 1) * HPC].swapaxes(1, 2)
            ).astype(NPMDT),
            "diagT": diagT,
        })
    return in_maps


def run(q, k, v, mask, Wq, Wk, Wv, Wo, **spmd_kwargs):
    nc = _build()
    in_maps = _prep_in_maps(q, k, v, mask, Wq, Wk, Wv, Wo)
    res = run_bass_kernel_spmd(nc, in_maps, list(range(NCORES)), **spmd_kwargs)
    outT = np.zeros((EMB, L), np.float64)
    for r in res.results:
        outT += r["outT"].astype(np.float64)
    out = np.ascontiguousarray(outT.T).astype(np.float32)
    return out, res


def kernel(q, k, v, mask, Wq, Wk, Wv, Wo):
    q, k, v = (np.asarray(x, np.float32) for x in (q, k, v))
    Wq, Wk, Wv, Wo = (np.asarray(x, np.float32) for x in (Wq, Wk, Wv, Wo))
    mask = np.asarray(mask, bool)
    out, _ = run(q, k, v, mask, Wq, Wk, Wv, Wo)
    return out
